# revision 15
# baseline (speedup 1.0000x reference)
"""DirGATv2Conv Trainium2 kernel (8 NeuronCores, SPMD).

Strategy: dst-tile edge sharding with mask-matmul segmented accumulation.
Core c owns target nodes [c*NPC, (c+1)*NPC) for both directions (direction 2
swaps src/dst roles). Edges are grouped host-side by destination node tile
(128 nodes); within a tile all per-node aggregation is done ON the tensor
engine with 0/1 mask matmuls accumulating in PSUM — no DRAM scatter-add, no
XR gather. The only per-edge DMA-gather is XL[src] (bf16 rows).

Per (tile t, direction d), E_t = G*128 edge slots:
  - dma_gather xlg = XL_d[src] (bf16, lo/hi split for the int16 index limit)
  - mask_accT[p,g,j] = (dstF[p,g]==j)   (DVE compare vs iota row)
  - dstRep = ones^T @ dstRow (PE) -> mask_ne[n,e] = (dstRep==n) (ACT+DVE)
  - per group g: PSUM m = mask_ne_g^T @ XR_t + eaT_g^T @ WeAug + I^T @ xlg_g
    (xr gather + edge-attr transform + xl add, all on PE), then
    mt = LeakyRelu(m) on ACT
  - score = reduce(mt * att) (DVE), a = exp(score) (ACT)
  - v = [a*xl | a] (DVE);  psum_acc += mask_accT_g^T @ v_g  (PE, per group)
  - after both directions: out rows = sum_d (num_d + bl_d*den_d)/(2 den_d+eps)
    + (bias1+bias2)/2

Softmax max-subtraction is skipped (alpha ratios are shift-invariant; scores
are O(1) here so exp cannot overflow). bf16 used for tables/masks/values
(rel tolerance 2e-2; observed error ~1e-3).
"""

import math
import sys

import numpy as np

# problem constants (hardcoded per harness contract)
N = 50000
E = 800000
D = 128
H = 4
CC = 32
HC = H * CC
ED = 16
ALPHA = 0.5
NEG_SLOPE = 0.2
NCORES = 8

LO_SPLIT = 32768      # int16 gather index limit
NPC = N // NCORES     # 6250 own nodes per core
NT = (NPC + 127) // 128  # 49 dst tiles per core
NPCP = NT * 128       # padded own rows


class Cfg:
    def __init__(self):
        self.N = N
        self.NPC = NPC
        self.NPCP = NPCP
        self.NT = NT
        # per direction: lists of per-tile lo/hi group counts (uniform across
        # cores), stream lengths, and per-tile start positions
        self.G_LO = [None, None]
        self.G_HI = [None, None]
        self.EP = [0, 0]
        self.POS = [None, None]
        self.G_MAX = 0


# ---------------------------------------------------------------------------
# host-side shard prep (pure numpy)
# ---------------------------------------------------------------------------

def _wrap_idx16(vals):
    """int16 index array in the [128, n/16] 16-partition-wrapped, 8x-replicated
    layout dma_gather expects."""
    v = np.asarray(vals, dtype=np.int16).reshape(-1, 16)   # [cols, 16]
    return np.tile(v.T, (8, 1))                            # [128, cols]


def prep_shards(inputs, cfg, ncores):
    """Returns (list of per-core input dicts, cfg filled in)."""
    x = np.asarray(inputs["x"], dtype=np.float32)
    ei = np.asarray(inputs["edge_index"])
    ea = np.asarray(inputs["edge_attr"], dtype=np.float32)

    import ml_dtypes
    bf16 = ml_dtypes.bfloat16

    xTb = np.ascontiguousarray(x.T).astype(bf16)                  # [128, N]
    per_core = [dict() for _ in range(ncores)]
    for c in range(ncores):
        xo = x[c * NPC:(c + 1) * NPC]
        xo_pad = np.zeros((NPCP, D), dtype=np.float32)
        xo_pad[:NPC] = xo
        per_core[c]["xTb"] = xTb
        per_core[c]["xToTb"] = np.ascontiguousarray(xo_pad.T).astype(bf16)

    Wl12 = np.concatenate([inputs["Wl1"], inputs["Wl2"]], axis=1).astype(np.float32)
    Wr12 = np.concatenate([inputs["Wr1"], inputs["Wr2"]], axis=1).astype(np.float32)
    iotaRow = np.tile(np.arange(128, dtype=np.float32)[None, :], (128, 1))
    iotaCol = np.arange(128, dtype=np.float32)[:, None]
    for c in range(ncores):
        per_core[c]["Wl12b"] = Wl12.astype(bf16)
        per_core[c]["Wr12b"] = Wr12.astype(bf16)
        per_core[c]["I128b"] = np.eye(128, dtype=np.float32).astype(bf16)
        per_core[c]["iotaRowB"] = iotaRow.astype(bf16)
        per_core[c]["iotaColB"] = iotaCol.astype(bf16)
        per_core[c]["onesB"] = np.ones((1, 128), dtype=np.float32).astype(bf16)
        for d, base in ((0, "1"), (1, "2")):
            # augmented ones-row carries the bl+br constant of the score path
            bsum = (np.asarray(inputs["bl" + base], dtype=np.float32)
                    + np.asarray(inputs["br" + base], dtype=np.float32))
            We_aug = np.concatenate(
                [np.asarray(inputs["We" + base], dtype=np.float32),
                 bsum[None, :]], axis=0)
            per_core[c][f"weAb{d}"] = We_aug.astype(bf16)          # [17, 128]
            att = np.asarray(inputs["att" + base], dtype=np.float32).reshape(1, HC)
            per_core[c][f"attB{d}"] = np.tile(att, (128, 1)).astype(bf16)
            bl = np.asarray(inputs["bl" + base], dtype=np.float32).reshape(1, HC)
            per_core[c][f"blB{d}"] = np.tile(bl, (128, 1))
        biasB = 0.5 * (np.asarray(inputs["bias1"], dtype=np.float32)
                       + np.asarray(inputs["bias2"], dtype=np.float32)).reshape(1, HC)
        per_core[c]["biasB"] = np.tile(biasB, (128, 1))

    # --- edge streams ------------------------------------------------------
    # per (direction, core): edges keyed by dst tile; within tile, lo sources
    # first then hi (int16 gather limit); each sub-run padded to 128 multiple
    # with group counts uniform across cores.
    for d in range(2):
        s_all = np.asarray(ei[0] if d == 0 else ei[1], dtype=np.int64)
        t_all = np.asarray(ei[1] if d == 0 else ei[0], dtype=np.int64)
        core_of = t_all // NPC

        counts = np.zeros((ncores, NT, 2), dtype=np.int64)
        per_core_edges = []
        for c in range(ncores):
            eids = np.flatnonzero(core_of == c)
            s = s_all[eids]
            t_rel = t_all[eids] - c * NPC
            tile = t_rel >> 7
            dloc = t_rel & 127
            hi = (s >= LO_SPLIT).astype(np.int64)
            key_cnt = np.bincount(tile * 2 + hi, minlength=NT * 2)
            counts[c] = np.stack([key_cnt[0::2], key_cnt[1::2]], axis=1)
            per_core_edges.append((eids, s, tile, dloc, hi))

        g_lo = np.maximum(1, (counts[:, :, 0].max(axis=0) + 127) // 128)
        g_hi = (counts[:, :, 1].max(axis=0) + 127) // 128
        G = g_lo + g_hi
        pos0 = np.zeros(NT, dtype=np.int64)
        pos0[1:] = np.cumsum(G[:-1] * 128)
        EP = int((G * 128).sum())
        cfg.G_LO[d] = g_lo.tolist()
        cfg.G_HI[d] = g_hi.tolist()
        cfg.EP[d] = EP
        cfg.POS[d] = pos0.tolist()
        cfg.G_MAX = max(cfg.G_MAX, int(G.max()))

        for c in range(ncores):
            eids, s, tile, dloc, hi = per_core_edges[c]
            # stable order by (tile, hi); rank within each (tile, hi) run
            key = tile * 2 + hi
            order = np.argsort(key, kind="stable")
            ks = key[order]
            starts = np.r_[0, np.flatnonzero(np.diff(ks)) + 1]
            seg_len = np.diff(np.r_[starts, len(ks)])
            rank = np.arange(len(ks)) - np.repeat(starts, seg_len)
            te = tile[order]
            he = hi[order]
            pos = pos0[te] + he * g_lo[te] * 128 + rank
            se = s[order] - he * LO_SPLIT
            de = dloc[order]
            ee = eids[order]

            sidx = np.zeros(EP, dtype=np.int64)
            sidx[pos] = se
            dstRow = np.full(EP, -1.0, dtype=np.float32)
            dstRow[pos] = de
            eaT = np.zeros((ED + 1, EP), dtype=np.float32)
            eaT[:ED, pos] = ea[ee].T
            eaT[ED, pos] = 1.0

            per_core[c][f"sidx{d}"] = _wrap_idx16(sidx)
            per_core[c][f"dstF{d}"] = \
                np.ascontiguousarray(dstRow.reshape(-1, 128).T).astype(bf16)
            per_core[c][f"dstRow{d}"] = dstRow[None, :].astype(bf16)
            per_core[c][f"eaT{d}"] = eaT.astype(bf16)
    return per_core, cfg


# ---------------------------------------------------------------------------
# device program
# ---------------------------------------------------------------------------

def build_program(cfg, nt_limit=None, skip=()):
    import concourse.bacc as bacc
    import concourse.bass as bass
    import concourse.mybir as mybir
    import concourse.tile as tile

    fp32 = mybir.dt.float32
    bf16 = mybir.dt.bfloat16
    i16 = mybir.dt.int16
    AF = mybir.ActivationFunctionType
    OP = mybir.AluOpType
    AX = mybir.AxisListType

    nc = bacc.Bacc("TRN2", target_bir_lowering=False)
    GM = cfg.G_MAX
    EPS2 = 2e-16

    # --- dram tensors -----------------------------------------------------
    xTb_t = nc.dram_tensor("xTb", [128, cfg.N], bf16, kind="ExternalInput")
    xToTb_t = nc.dram_tensor("xToTb", [128, cfg.NPCP], bf16, kind="ExternalInput")
    Wl12b_t = nc.dram_tensor("Wl12b", [128, 256], bf16, kind="ExternalInput")
    Wr12b_t = nc.dram_tensor("Wr12b", [128, 256], bf16, kind="ExternalInput")
    I128b_t = nc.dram_tensor("I128b", [128, 128], bf16, kind="ExternalInput")
    iotaRowB_t = nc.dram_tensor("iotaRowB", [128, 128], bf16, kind="ExternalInput")
    iotaColB_t = nc.dram_tensor("iotaColB", [128, 1], bf16, kind="ExternalInput")
    onesB_t = nc.dram_tensor("onesB", [1, 128], bf16, kind="ExternalInput")
    biasB_t = nc.dram_tensor("biasB", [128, HC], fp32, kind="ExternalInput")
    weAb_t, attB_t, blB_t, sidx_t, dstF_t, dstRow_t, eaT_t = \
        [], [], [], [], [], [], []
    for d in range(2):
        ep = cfg.EP[d]
        weAb_t.append(nc.dram_tensor(f"weAb{d}", [ED + 1, HC], bf16, kind="ExternalInput"))
        attB_t.append(nc.dram_tensor(f"attB{d}", [128, HC], bf16, kind="ExternalInput"))
        blB_t.append(nc.dram_tensor(f"blB{d}", [128, HC], fp32, kind="ExternalInput"))
        sidx_t.append(nc.dram_tensor(f"sidx{d}", [128, ep // 16], i16, kind="ExternalInput"))
        dstF_t.append(nc.dram_tensor(f"dstF{d}", [128, ep // 128], bf16, kind="ExternalInput"))
        dstRow_t.append(nc.dram_tensor(f"dstRow{d}", [1, ep], bf16, kind="ExternalInput"))
        eaT_t.append(nc.dram_tensor(f"eaT{d}", [ED + 1, ep], bf16, kind="ExternalInput"))
    out_t = nc.dram_tensor("out", [cfg.NPC, HC], fp32, kind="ExternalOutput")

    XLb = [nc.dram_tensor(f"XLb{d}", [cfg.N, HC], bf16, kind="Internal")
           for d in range(2)]

    with tile.TileContext(nc) as tc:
        # persistent constants + XR table in SBUF
        with tc.tile_pool(name="wp", bufs=1) as wp:
            i128_sb = wp.tile([128, 128], bf16, tag="i128")
            nc.sync.dma_start(out=i128_sb[:], in_=I128b_t[:])
            iotar_sb = wp.tile([128, 128], bf16, tag="iotar")
            nc.sync.dma_start(out=iotar_sb[:], in_=iotaRowB_t[:])
            iotac_sb = wp.tile([128, 1], bf16, tag="iotac")
            nc.sync.dma_start(out=iotac_sb[:], in_=iotaColB_t[:])
            ones_sb = wp.tile([1, 128], bf16, tag="ones")
            nc.sync.dma_start(out=ones_sb[:], in_=onesB_t[:])
            bias_sb = wp.tile([128, HC], fp32, tag="bias")
            nc.sync.dma_start(out=bias_sb[:], in_=biasB_t[:])
            we_sb, att_sb, bl_sb = [], [], []
            for d in range(2):
                t_ = wp.tile([ED + 1, HC], bf16, tag=f"we{d}", name=f"we{d}")
                nc.sync.dma_start(out=t_[:], in_=weAb_t[d][:])
                we_sb.append(t_)
                t_ = wp.tile([128, HC], bf16, tag=f"att{d}", name=f"att{d}")
                nc.sync.dma_start(out=t_[:], in_=attB_t[d][:])
                att_sb.append(t_)
                t_ = wp.tile([128, HC], fp32, tag=f"bl{d}", name=f"bl{d}")
                nc.sync.dma_start(out=t_[:], in_=blB_t[d][:])
                bl_sb.append(t_)
            wl_sb = wp.tile([128, 256], bf16, tag="wl")
            nc.sync.dma_start(out=wl_sb[:], in_=Wl12b_t[:])
            wr_sb = wp.tile([128, 256], bf16, tag="wr")
            nc.sync.dma_start(out=wr_sb[:], in_=Wr12b_t[:])
            # XR table lives in SBUF: [128 node, NT*256] (per tile: d0|d1)
            xr_all = wp.tile([128, cfg.NT * 256], bf16, tag="xr_all")

            # --- phase 1: XL tables (DRAM) + XR table (SBUF) --------------
            with (tc.tile_pool(name="tb_sb", bufs=3) as sp,
                  tc.tile_pool(name="tb_ps", bufs=2, space="PSUM") as pp):
                ntile = (cfg.N + 127) // 128
                for i in range(ntile):
                    r = min(128, cfg.N - i * 128)
                    xt_sb = sp.tile([128, 128], bf16, tag="xt")
                    nc.sync.dma_start(out=xt_sb[:, :r], in_=xTb_t[:, i * 128:i * 128 + r])
                    ps = pp.tile([128, 256], fp32, tag="tps")
                    nc.tensor.matmul(out=ps[:r, :], lhsT=xt_sb[:, :r], rhs=wl_sb[:],
                                     start=True, stop=True)
                    ev = sp.tile([128, 256], bf16, tag="ev")
                    nc.vector.tensor_copy(out=ev[:r, :128], in_=ps[:r, :128])
                    nc.scalar.copy(out=ev[:r, 128:], in_=ps[:r, 128:])
                    nc.sync.dma_start(out=XLb[0][i * 128:i * 128 + r, :], in_=ev[:r, :128])
                    nc.sync.dma_start(out=XLb[1][i * 128:i * 128 + r, :], in_=ev[:r, 128:])
                for i in range(cfg.NT):
                    xt_sb = sp.tile([128, 128], bf16, tag="xt")
                    nc.sync.dma_start(out=xt_sb[:], in_=xToTb_t[:, i * 128:(i + 1) * 128])
                    ps = pp.tile([128, 256], fp32, tag="tps")
                    nc.tensor.matmul(out=ps[:], lhsT=xt_sb[:], rhs=wr_sb[:],
                                     start=True, stop=True)
                    nc.vector.tensor_copy(out=xr_all[:, i * 256:i * 256 + 128],
                                          in_=ps[:, :128])
                    nc.scalar.copy(out=xr_all[:, i * 256 + 128:i * 256 + 256],
                                   in_=ps[:, 128:])

            # --- phase 2: per (tile, direction) edge processing -----------
            with (tc.tile_pool(name="ed_sb", bufs=2) as ep_,
                  tc.tile_pool(name="ed_ps", bufs=3, space="PSUM") as ppA,
                  tc.tile_pool(name="rep_ps", bufs=2, space="PSUM") as ppR,
                  tc.tile_pool(name="acc_ps", bufs=1, space="PSUM") as ppC,
                  tc.tile_pool(name="fn_sb", bufs=3) as fp_):
                for t in range(cfg.NT if nt_limit is None else nt_limit):
                    nm = [None, None]
                    for d in range(2):
                        glo, ghi = cfg.G_LO[d][t], cfg.G_HI[d][t]
                        G = glo + ghi
                        Et = G * 128
                        base = cfg.POS[d][t]

                        sidx_sb = ep_.tile([128, GM * 8], i16, tag="sidx")
                        nc.sync.dma_start(
                            out=sidx_sb[:, :G * 8],
                            in_=sidx_t[d][:, base // 16:(base + Et) // 16])
                        dstF_sb = ep_.tile([128, GM], bf16, tag="dstF")
                        nc.sync.dma_start(
                            out=dstF_sb[:, :G],
                            in_=dstF_t[d][:, base // 128:base // 128 + G])
                        dstR_sb = ep_.tile([1, GM * 128], bf16, tag="dstR")
                        nc.sync.dma_start(out=dstR_sb[:, :Et],
                                          in_=dstRow_t[d][:, base:base + Et])
                        eaT_sb = ep_.tile([ED + 1, GM * 128], bf16, tag="ea")
                        nc.sync.dma_start(out=eaT_sb[:, :Et],
                                          in_=eaT_t[d][:, base:base + Et])

                        xlg = ep_.tile([128, GM, 128], bf16, tag="xlg")
                        if "gather" in skip:
                            nc.vector.memset(xlg[:, :G, :], 0)
                        else:
                            # hardware caps one gather call at 1024 indices
                            for g0, g1, src in ((0, glo, XLb[d][:]),
                                                (glo, G, XLb[d][LO_SPLIT:, :])):
                                g_ = g0
                                while g_ < g1:
                                    cnt = min(8, g1 - g_)
                                    nc.gpsimd.dma_gather(
                                        out_ap=xlg[:, g_:g_ + cnt, :], in_ap=src,
                                        idxs_ap=sidx_sb[:, g_ * 8:(g_ + cnt) * 8],
                                        num_idxs=cnt * 128,
                                        num_idxs_reg=cnt * 128,
                                        elem_size=HC)
                                    g_ += cnt

                        # mask_accT[p, g, j] = (dstF[p, g] == j)
                        mask_acc = ep_.tile([128, GM, 128], bf16, tag="macc")
                        dstF_ap = dstF_sb[:, :G]
                        dstF_bc = bass.AP(dstF_ap.tensor, dstF_ap.offset,
                                          [dstF_ap.ap[0], dstF_ap.ap[1], [0, 128]])
                        iot_ap = iotar_sb[:]
                        iot_bc = bass.AP(iot_ap.tensor, iot_ap.offset,
                                         [iot_ap.ap[0], [0, G], iot_ap.ap[1]])
                        nc.vector.tensor_tensor(out=mask_acc[:, :G, :],
                                                in0=dstF_bc, in1=iot_bc,
                                                op=OP.is_equal)

                        # mask_ne[n, e] = (dst(e) == n), via PE replication
                        mask_ne = ep_.tile([128, GM * 128], bf16, tag="mne")
                        nsl = 0 if "rep" in skip else (Et + 511) // 512
                        if "rep" in skip:
                            nc.vector.memset(mask_ne[:, :Et], 0)
                        for s_ in range(nsl):
                            w = min(512, Et - s_ * 512)
                            psr = ppR.tile([128, 512], fp32, tag="psr")
                            nc.tensor.matmul(
                                out=psr[:, :w], lhsT=ones_sb[:],
                                rhs=dstR_sb[:, s_ * 512:s_ * 512 + w],
                                start=True, stop=True)
                            ic_ap = iotac_sb[:]
                            ic_bc = bass.AP(ic_ap.tensor, ic_ap.offset,
                                            [ic_ap.ap[0], [0, w]])
                            nc.vector.tensor_tensor(
                                out=mask_ne[:, s_ * 512:s_ * 512 + w],
                                in0=psr[:, :w], in1=ic_bc, op=OP.is_equal)

                        # per group: m = xr[dst] + ea@We + xl  (PSUM), then
                        # LeakyRelu on ACT
                        mt = ep_.tile([128, GM, 128], bf16, tag="mt")
                        if "grp" in skip:
                            nc.vector.memset(mt[:, :G, :], 0)
                        else:
                            for g in range(G):
                                psA = ppA.tile([128, 128], fp32, tag="psA")
                                nc.tensor.matmul(
                                    out=psA[:], lhsT=mask_ne[:, g * 128:(g + 1) * 128],
                                    rhs=xr_all[:, t * 256 + d * 128:t * 256 + d * 128 + 128],
                                    start=True, stop=False)
                                nc.tensor.matmul(
                                    out=psA[:], lhsT=eaT_sb[:, g * 128:(g + 1) * 128],
                                    rhs=we_sb[d][:], start=False, stop=False)
                                nc.tensor.matmul(
                                    out=psA[:], lhsT=i128_sb[:], rhs=xlg[:, g, :],
                                    start=False, stop=True)
                                nc.scalar.copy(out=mt[:, g, :], in_=psA[:])
                            # leaky_relu(x) = max(0.2x, x), one DVE op on bf16
                            nc.vector.scalar_tensor_tensor(
                                out=mt[:, :G, :], in0=mt[:, :G, :],
                                scalar=NEG_SLOPE, in1=mt[:, :G, :],
                                op0=OP.mult, op1=OP.max)

                        # score + exp
                        mm = ep_.tile([128, GM, 128], bf16, tag="mm")
                        att_ap = att_sb[d][:]
                        att_bc = bass.AP(att_ap.tensor, att_ap.offset,
                                         [att_ap.ap[0], [0, G], att_ap.ap[1]])
                        nc.vector.tensor_tensor(out=mm[:, :G, :], in0=mt[:, :G, :],
                                                in1=att_bc, op=OP.mult)
                        sc = ep_.tile([128, GM, H], fp32, tag="sc")
                        nc.vector.tensor_reduce(
                            out=sc[:, :G, :],
                            in_=mm[:, :G, :].rearrange("p g (h c) -> p g h c", h=H),
                            axis=AX.X, op=OP.add)
                        av = ep_.tile([128, GM, H], bf16, tag="av")
                        nc.scalar.activation(out=av[:, :G, :], in_=sc[:, :G, :],
                                             func=AF.Exp)

                        # v = [a*xl | a]
                        v = ep_.tile([128, GM, 132], bf16, tag="v")
                        av_ap = av[:, :G, :]
                        av_bc = bass.AP(av_ap.tensor, av_ap.offset,
                                        [av_ap.ap[0], av_ap.ap[1], av_ap.ap[2],
                                         [0, CC]])
                        nc.vector.tensor_tensor(
                            out=v[:, :G, 0:128].rearrange("p g (h c) -> p g h c", h=H),
                            in0=xlg[:, :G, :].rearrange("p g (h c) -> p g h c", h=H),
                            in1=av_bc, op=OP.mult)
                        nc.scalar.copy(out=v[:, :G, 128:132], in_=av[:, :G, :])

                        # segmented accumulation into PSUM
                        nm_sb = fp_.tile([128, 132], fp32, tag=f"nm{d}",
                                         name=f"nm{d}")
                        if "acc" in skip:
                            nc.vector.memset(nm_sb[:], 0)
                        else:
                            psC = ppC.tile([128, 132], fp32, tag=f"acc{d}",
                                           name=f"acc{d}")
                            for g in range(G):
                                nc.tensor.matmul(
                                    out=psC[:], lhsT=mask_acc[:, g, :],
                                    rhs=v[:, g, :], start=(g == 0),
                                    stop=(g == G - 1))
                            nc.scalar.copy(out=nm_sb[:], in_=psC[:])
                        nm[d] = nm_sb

                    # --- combine both directions, write own rows ----------
                    rows = min(128, cfg.NPC - t * 128)
                    acc = fp_.tile([128, HC], fp32, tag="acc")
                    nc.vector.tensor_copy(out=acc[:], in_=bias_sb[:])
                    for d in range(2):
                        num_sb = nm[d][:, 0:128]
                        den_sb = nm[d][:, 128:132]
                        d2 = fp_.tile([128, H], fp32, tag="d2")
                        nc.vector.tensor_scalar(out=d2[:], in0=den_sb,
                                                scalar1=2.0, scalar2=EPS2,
                                                op0=OP.mult, op1=OP.add)
                        rd = fp_.tile([128, H], fp32, tag="rd")
                        nc.vector.reciprocal(out=rd[:], in_=d2[:])
                        rd_ap = rd[:]
                        rd_bc = bass.AP(rd_ap.tensor, rd_ap.offset,
                                        [rd_ap.ap[0], rd_ap.ap[1], [0, CC]])
                        t_d = fp_.tile([128, HC], fp32, tag="t_d")
                        nc.vector.tensor_tensor(
                            out=t_d[:].rearrange("p (h c) -> p h c", h=H),
                            in0=num_sb.rearrange("p (h c) -> p h c", h=H),
                            in1=rd_bc, op=OP.mult)
                        nc.vector.tensor_add(out=acc[:], in0=acc[:], in1=t_d[:])
                        ds = fp_.tile([128, H], fp32, tag="ds")
                        nc.vector.tensor_mul(out=ds[:], in0=den_sb, in1=rd[:])
                        ds_ap = ds[:]
                        ds_bc = bass.AP(ds_ap.tensor, ds_ap.offset,
                                        [ds_ap.ap[0], ds_ap.ap[1], [0, CC]])
                        u_d = fp_.tile([128, HC], fp32, tag="u_d")
                        nc.vector.tensor_tensor(
                            out=u_d[:].rearrange("p (h c) -> p h c", h=H),
                            in0=bl_sb[d][:].rearrange("p (h c) -> p h c", h=H),
                            in1=ds_bc, op=OP.mult)
                        nc.vector.tensor_add(out=acc[:], in0=acc[:], in1=u_d[:])
                    nc.sync.dma_start(out=out_t[t * 128:t * 128 + rows, :],
                                      in_=acc[:rows, :])

    nc.compile()
    return nc


# ---------------------------------------------------------------------------
# entry point
# ---------------------------------------------------------------------------

def kernel(**inputs):
    for p in ("/opt/trn_rl_repo",):
        if p not in sys.path:
            sys.path.insert(0, p)
    from concourse.bass_utils import run_bass_kernel_spmd

    cfg = Cfg()
    shards, cfg = prep_shards(inputs, cfg, NCORES)
    nc = build_program(cfg)
    res = run_bass_kernel_spmd(nc, shards, core_ids=list(range(NCORES)))
    out = np.concatenate([res.results[c]["out"] for c in range(NCORES)], axis=0)
    return out.astype(np.float32)


# revision 19
# speedup vs baseline: 1.5415x; 1.5415x over previous
"""DirGATv2Conv Trainium2 kernel (8 NeuronCores, SPMD).

Strategy: dst-tile edge sharding with mask-matmul segmented accumulation.
Core c owns target nodes [c*NPC, (c+1)*NPC) for both directions (direction 2
swaps src/dst roles). Edges are grouped host-side by destination node tile
(128 nodes); within a tile all per-node aggregation is done ON the tensor
engine with 0/1 mask matmuls accumulating in PSUM — no DRAM scatter-add, no
XR gather. The only per-edge DMA-gather is XL[src] (bf16 rows).

Per (tile t, direction d), E_t = G*128 edge slots:
  - dma_gather xlg = XL_d[src] (bf16, lo/hi split for the int16 index limit)
  - mask_accT[p,g,j] = (dstF[p,g]==j)   (DVE compare vs iota row)
  - dstRep = ones^T @ dstRow (PE) -> mask_ne[n,e] = (dstRep==n) (ACT+DVE)
  - per group g: PSUM m = mask_ne_g^T @ XR_t + eaT_g^T @ WeAug + I^T @ xlg_g
    (xr gather + edge-attr transform + xl add, all on PE), then
    mt = LeakyRelu(m) on ACT
  - score = reduce(mt * att) (DVE), a = exp(score) (ACT)
  - v = [a*xl | a] (DVE);  psum_acc += mask_accT_g^T @ v_g  (PE, per group)
  - after both directions: out rows = sum_d (num_d + bl_d*den_d)/(2 den_d+eps)
    + (bias1+bias2)/2

Softmax max-subtraction is skipped (alpha ratios are shift-invariant; scores
are O(1) here so exp cannot overflow). bf16 used for tables/masks/values
(rel tolerance 2e-2; observed error ~1e-3).
"""

import math
import sys

import numpy as np

# problem constants (hardcoded per harness contract)
N = 50000
E = 800000
D = 128
H = 4
CC = 32
HC = H * CC
ED = 16
ALPHA = 0.5
NEG_SLOPE = 0.2
NCORES = 8

LO_SPLIT = 32768      # int16 gather index limit
NPC = N // NCORES     # 6250 own nodes per core
NT = (NPC + 127) // 128  # 49 dst tiles per core
NPCP = NT * 128       # padded own rows


class Cfg:
    def __init__(self):
        self.N = N
        self.NPC = NPC
        self.NPCP = NPCP
        self.NT = NT
        # per direction: lists of per-tile lo/hi group counts (uniform across
        # cores), stream lengths, and per-tile start positions
        self.G_LO = [None, None]
        self.G_HI = [None, None]
        self.EP = [0, 0]
        self.POS = [None, None]
        self.G_MAX = 0


# ---------------------------------------------------------------------------
# host-side shard prep (pure numpy)
# ---------------------------------------------------------------------------

def _wrap_idx16(vals):
    """int16 index array in the [128, n/16] 16-partition-wrapped, 8x-replicated
    layout dma_gather expects."""
    v = np.asarray(vals, dtype=np.int16).reshape(-1, 16)   # [cols, 16]
    return np.tile(v.T, (8, 1))                            # [128, cols]


def prep_shards(inputs, cfg, ncores):
    """Returns (list of per-core input dicts, cfg filled in)."""
    x = np.asarray(inputs["x"], dtype=np.float32)
    ei = np.asarray(inputs["edge_index"])
    ea = np.asarray(inputs["edge_attr"], dtype=np.float32)

    import ml_dtypes
    bf16 = ml_dtypes.bfloat16

    xTb = np.ascontiguousarray(x.T).astype(bf16)                  # [128, N]
    per_core = [dict() for _ in range(ncores)]
    for c in range(ncores):
        xo = x[c * NPC:(c + 1) * NPC]
        xo_pad = np.zeros((NPCP, D), dtype=np.float32)
        xo_pad[:NPC] = xo
        per_core[c]["xTb"] = xTb
        per_core[c]["xToTb"] = np.ascontiguousarray(xo_pad.T).astype(bf16)

    Wl12 = np.concatenate([inputs["Wl1"], inputs["Wl2"]], axis=1).astype(np.float32)
    Wr12 = np.concatenate([inputs["Wr1"], inputs["Wr2"]], axis=1).astype(np.float32)
    iotaRow = np.tile(np.arange(128, dtype=np.float32)[None, :], (128, 1))
    iotaCol = np.arange(128, dtype=np.float32)[:, None]
    for c in range(ncores):
        per_core[c]["Wl12b"] = Wl12.astype(bf16)
        per_core[c]["Wr12b"] = Wr12.astype(bf16)
        per_core[c]["I128b"] = np.eye(128, dtype=np.float32).astype(bf16)
        per_core[c]["iotaRowB"] = iotaRow.astype(bf16)
        per_core[c]["iotaColB"] = iotaCol.astype(bf16)
        per_core[c]["onesB"] = np.ones((1, 128), dtype=np.float32).astype(bf16)
        for d, base in ((0, "1"), (1, "2")):
            # augmented ones-row carries the bl+br constant of the score path
            bsum = (np.asarray(inputs["bl" + base], dtype=np.float32)
                    + np.asarray(inputs["br" + base], dtype=np.float32))
            We_aug = np.concatenate(
                [np.asarray(inputs["We" + base], dtype=np.float32),
                 bsum[None, :]], axis=0)
            per_core[c][f"weAb{d}"] = We_aug.astype(bf16)          # [17, 128]
            att = np.asarray(inputs["att" + base], dtype=np.float32).reshape(1, HC)
            per_core[c][f"attB{d}"] = np.tile(att, (128, 1)).astype(bf16)
            bl = np.asarray(inputs["bl" + base], dtype=np.float32).reshape(1, HC)
            per_core[c][f"blB{d}"] = np.tile(bl, (128, 1))
        biasB = 0.5 * (np.asarray(inputs["bias1"], dtype=np.float32)
                       + np.asarray(inputs["bias2"], dtype=np.float32)).reshape(1, HC)
        per_core[c]["biasB"] = np.tile(biasB, (128, 1))

    # --- edge streams ------------------------------------------------------
    # per (direction, core): edges keyed by dst tile; within tile, lo sources
    # first then hi (int16 gather limit); each sub-run padded to 128 multiple
    # with group counts uniform across cores.
    for d in range(2):
        s_all = np.asarray(ei[0] if d == 0 else ei[1], dtype=np.int64)
        t_all = np.asarray(ei[1] if d == 0 else ei[0], dtype=np.int64)
        core_of = t_all // NPC

        counts = np.zeros((ncores, NT, 2), dtype=np.int64)
        per_core_edges = []
        for c in range(ncores):
            eids = np.flatnonzero(core_of == c)
            s = s_all[eids]
            t_rel = t_all[eids] - c * NPC
            tile = t_rel >> 7
            dloc = t_rel & 127
            hi = (s >= LO_SPLIT).astype(np.int64)
            key_cnt = np.bincount(tile * 2 + hi, minlength=NT * 2)
            counts[c] = np.stack([key_cnt[0::2], key_cnt[1::2]], axis=1)
            per_core_edges.append((eids, s, tile, dloc, hi))

        g_lo = np.maximum(1, (counts[:, :, 0].max(axis=0) + 127) // 128)
        g_hi = (counts[:, :, 1].max(axis=0) + 127) // 128
        G = g_lo + g_hi
        pos0 = np.zeros(NT, dtype=np.int64)
        pos0[1:] = np.cumsum(G[:-1] * 128)
        EP = int((G * 128).sum())
        cfg.G_LO[d] = g_lo.tolist()
        cfg.G_HI[d] = g_hi.tolist()
        cfg.EP[d] = EP
        cfg.POS[d] = pos0.tolist()
        cfg.G_MAX = max(cfg.G_MAX, int(G.max()))

        for c in range(ncores):
            eids, s, tile, dloc, hi = per_core_edges[c]
            # stable order by (tile, hi); rank within each (tile, hi) run
            key = tile * 2 + hi
            order = np.argsort(key, kind="stable")
            ks = key[order]
            starts = np.r_[0, np.flatnonzero(np.diff(ks)) + 1]
            seg_len = np.diff(np.r_[starts, len(ks)])
            rank = np.arange(len(ks)) - np.repeat(starts, seg_len)
            te = tile[order]
            he = hi[order]
            pos = pos0[te] + he * g_lo[te] * 128 + rank
            se = s[order] - he * LO_SPLIT
            de = dloc[order]
            ee = eids[order]

            sidx = np.zeros(EP, dtype=np.int64)
            sidx[pos] = se
            dstRow = np.full(EP, -1.0, dtype=np.float32)
            dstRow[pos] = de
            eaT = np.zeros((ED + 1, EP), dtype=np.float32)
            eaT[:ED, pos] = ea[ee].T
            eaT[ED, pos] = 1.0

            per_core[c][f"sidx{d}"] = _wrap_idx16(sidx)
            per_core[c][f"dstF{d}"] = \
                np.ascontiguousarray(dstRow.reshape(-1, 128).T).astype(bf16)
            per_core[c][f"dstRow{d}"] = dstRow[None, :].astype(bf16)
            per_core[c][f"eaT{d}"] = eaT.astype(bf16)
    return per_core, cfg


# ---------------------------------------------------------------------------
# device program
# ---------------------------------------------------------------------------

def build_program(cfg, nt_limit=None, skip=()):
    import concourse.bacc as bacc
    import concourse.bass as bass
    import concourse.mybir as mybir
    import concourse.tile as tile

    fp32 = mybir.dt.float32
    bf16 = mybir.dt.bfloat16
    i16 = mybir.dt.int16
    AF = mybir.ActivationFunctionType
    OP = mybir.AluOpType
    AX = mybir.AxisListType

    nc = bacc.Bacc("TRN2", target_bir_lowering=False)
    GM = cfg.G_MAX
    EPS2 = 2e-16

    # --- dram tensors -----------------------------------------------------
    xTb_t = nc.dram_tensor("xTb", [128, cfg.N], bf16, kind="ExternalInput")
    xToTb_t = nc.dram_tensor("xToTb", [128, cfg.NPCP], bf16, kind="ExternalInput")
    Wl12b_t = nc.dram_tensor("Wl12b", [128, 256], bf16, kind="ExternalInput")
    Wr12b_t = nc.dram_tensor("Wr12b", [128, 256], bf16, kind="ExternalInput")
    I128b_t = nc.dram_tensor("I128b", [128, 128], bf16, kind="ExternalInput")
    iotaRowB_t = nc.dram_tensor("iotaRowB", [128, 128], bf16, kind="ExternalInput")
    iotaColB_t = nc.dram_tensor("iotaColB", [128, 1], bf16, kind="ExternalInput")
    onesB_t = nc.dram_tensor("onesB", [1, 128], bf16, kind="ExternalInput")
    biasB_t = nc.dram_tensor("biasB", [128, HC], fp32, kind="ExternalInput")
    weAb_t, attB_t, blB_t, sidx_t, dstF_t, dstRow_t, eaT_t = \
        [], [], [], [], [], [], []
    for d in range(2):
        ep = cfg.EP[d]
        weAb_t.append(nc.dram_tensor(f"weAb{d}", [ED + 1, HC], bf16, kind="ExternalInput"))
        attB_t.append(nc.dram_tensor(f"attB{d}", [128, HC], bf16, kind="ExternalInput"))
        blB_t.append(nc.dram_tensor(f"blB{d}", [128, HC], fp32, kind="ExternalInput"))
        sidx_t.append(nc.dram_tensor(f"sidx{d}", [128, ep // 16], i16, kind="ExternalInput"))
        dstF_t.append(nc.dram_tensor(f"dstF{d}", [128, ep // 128], bf16, kind="ExternalInput"))
        dstRow_t.append(nc.dram_tensor(f"dstRow{d}", [1, ep], bf16, kind="ExternalInput"))
        eaT_t.append(nc.dram_tensor(f"eaT{d}", [ED + 1, ep], bf16, kind="ExternalInput"))
    out_t = nc.dram_tensor("out", [cfg.NPC, HC], fp32, kind="ExternalOutput")

    XLb = [nc.dram_tensor(f"XLb{d}", [cfg.N, HC], bf16, kind="Internal")
           for d in range(2)]

    with tile.TileContext(nc) as tc:
        # persistent constants + XR table in SBUF
        with tc.tile_pool(name="wp", bufs=1) as wp:
            i128_sb = wp.tile([128, 128], bf16, tag="i128")
            nc.sync.dma_start(out=i128_sb[:], in_=I128b_t[:])
            iotar_sb = wp.tile([128, 128], bf16, tag="iotar")
            nc.sync.dma_start(out=iotar_sb[:], in_=iotaRowB_t[:])
            iotac_sb = wp.tile([128, 1], bf16, tag="iotac")
            nc.sync.dma_start(out=iotac_sb[:], in_=iotaColB_t[:])
            ones_sb = wp.tile([1, 128], bf16, tag="ones")
            nc.sync.dma_start(out=ones_sb[:], in_=onesB_t[:])
            bias_sb = wp.tile([128, HC], fp32, tag="bias")
            nc.sync.dma_start(out=bias_sb[:], in_=biasB_t[:])
            we_sb, att_sb, bl_sb = [], [], []
            for d in range(2):
                t_ = wp.tile([ED + 1, HC], bf16, tag=f"we{d}", name=f"we{d}")
                nc.sync.dma_start(out=t_[:], in_=weAb_t[d][:])
                we_sb.append(t_)
                t_ = wp.tile([128, HC], bf16, tag=f"att{d}", name=f"att{d}")
                nc.sync.dma_start(out=t_[:], in_=attB_t[d][:])
                att_sb.append(t_)
                t_ = wp.tile([128, HC], fp32, tag=f"bl{d}", name=f"bl{d}")
                nc.sync.dma_start(out=t_[:], in_=blB_t[d][:])
                bl_sb.append(t_)
            wl_sb = wp.tile([128, 256], bf16, tag="wl")
            nc.sync.dma_start(out=wl_sb[:], in_=Wl12b_t[:])
            wr_sb = wp.tile([128, 256], bf16, tag="wr")
            nc.sync.dma_start(out=wr_sb[:], in_=Wr12b_t[:])
            # XR table lives in SBUF: [128 node, NT*256] (per tile: d0|d1)
            xr_all = wp.tile([128, cfg.NT * 256], bf16, tag="xr_all")

            # --- phase 1: XL tables (DRAM) + XR table (SBUF) --------------
            # batched 4 node-tiles per DMA round-trip to keep PE fed
            with (tc.tile_pool(name="tb_sb", bufs=4) as sp,
                  tc.tile_pool(name="tb_ps", bufs=4, space="PSUM") as pp):
                nfull = cfg.N // 512
                for i in range(nfull + 1):
                    r0 = i * 512
                    rem = min(512, cfg.N - r0)
                    if rem <= 0:
                        break
                    nt4 = (rem + 127) // 128
                    xt4 = sp.tile([128, 512], bf16, tag="xt4")
                    nc.sync.dma_start(out=xt4[:, :rem], in_=xTb_t[:, r0:r0 + rem])
                    ev4 = sp.tile([128, 4, 256], bf16, tag="ev4")
                    for j in range(nt4):
                        rr = min(128, rem - j * 128)
                        ps = pp.tile([128, 256], fp32, tag="tps")
                        nc.tensor.matmul(out=ps[:rr, :],
                                         lhsT=xt4[:, j * 128:j * 128 + rr],
                                         rhs=wl_sb[:], start=True, stop=True)
                        nc.vector.tensor_copy(out=ev4[:rr, j, 0:128], in_=ps[:rr, :128])
                        nc.scalar.copy(out=ev4[:rr, j, 128:256], in_=ps[:rr, 128:])
                    if rem == 512:
                        for d in range(2):
                            nc.sync.dma_start(
                                out=XLb[d][r0:r0 + 512, :].rearrange(
                                    "(j p) c -> p j c", j=4),
                                in_=ev4[:, :, d * 128:d * 128 + 128])
                    else:
                        for j in range(nt4):
                            rr = min(128, rem - j * 128)
                            for d in range(2):
                                nc.sync.dma_start(
                                    out=XLb[d][r0 + j * 128:r0 + j * 128 + rr, :],
                                    in_=ev4[:rr, j, d * 128:d * 128 + 128])
                for i in range(cfg.NT):
                    xt_sb = sp.tile([128, 128], bf16, tag="xt")
                    nc.sync.dma_start(out=xt_sb[:], in_=xToTb_t[:, i * 128:(i + 1) * 128])
                    ps = pp.tile([128, 256], fp32, tag="tps")
                    nc.tensor.matmul(out=ps[:], lhsT=xt_sb[:], rhs=wr_sb[:],
                                     start=True, stop=True)
                    nc.vector.tensor_copy(out=xr_all[:, i * 256:i * 256 + 128],
                                          in_=ps[:, :128])
                    nc.scalar.copy(out=xr_all[:, i * 256 + 128:i * 256 + 256],
                                   in_=ps[:, 128:])

            # --- phase 2: per (tile, direction) edge processing -----------
            with (tc.tile_pool(name="g_sb", bufs=3) as gp_,
                  tc.tile_pool(name="ed_sb", bufs=3) as ep_,
                  tc.tile_pool(name="ed_ps", bufs=3, space="PSUM") as ppA,
                  tc.tile_pool(name="rep_ps", bufs=2, space="PSUM") as ppR,
                  tc.tile_pool(name="acc_ps", bufs=1, space="PSUM") as ppC,
                  tc.tile_pool(name="fn_sb", bufs=3) as fp_):
                for t in range(cfg.NT if nt_limit is None else nt_limit):
                    nm = [None, None]
                    for d in range(2):
                        glo, ghi = cfg.G_LO[d][t], cfg.G_HI[d][t]
                        G = glo + ghi
                        Et = G * 128
                        base = cfg.POS[d][t]

                        sidx_sb = gp_.tile([128, GM * 8], i16, tag="sidx")
                        nc.sync.dma_start(
                            out=sidx_sb[:, :G * 8],
                            in_=sidx_t[d][:, base // 16:(base + Et) // 16])
                        dstF_sb = ep_.tile([128, GM], bf16, tag="dstF")
                        nc.sync.dma_start(
                            out=dstF_sb[:, :G],
                            in_=dstF_t[d][:, base // 128:base // 128 + G])
                        dstR_sb = ep_.tile([1, GM * 128], bf16, tag="dstR")
                        nc.sync.dma_start(out=dstR_sb[:, :Et],
                                          in_=dstRow_t[d][:, base:base + Et])
                        eaT_sb = ep_.tile([ED + 1, GM * 128], bf16, tag="ea")
                        nc.sync.dma_start(out=eaT_sb[:, :Et],
                                          in_=eaT_t[d][:, base:base + Et])

                        xlg = gp_.tile([128, GM, 128], bf16, tag="xlg")
                        if "gather" in skip:
                            nc.vector.memset(xlg[:, :G, :], 0)
                        else:
                            # hardware caps one gather call at 1024 indices
                            for g0, g1, src in ((0, glo, XLb[d][:]),
                                                (glo, G, XLb[d][LO_SPLIT:, :])):
                                g_ = g0
                                while g_ < g1:
                                    cnt = min(8, g1 - g_)
                                    nc.gpsimd.dma_gather(
                                        out_ap=xlg[:, g_:g_ + cnt, :], in_ap=src,
                                        idxs_ap=sidx_sb[:, g_ * 8:(g_ + cnt) * 8],
                                        num_idxs=cnt * 128,
                                        num_idxs_reg=cnt * 128,
                                        elem_size=HC)
                                    g_ += cnt

                        # mask_accT[p, g, j] = (dstF[p, g] == j)
                        mask_acc = ep_.tile([128, GM, 128], bf16, tag="macc")
                        dstF_ap = dstF_sb[:, :G]
                        dstF_bc = bass.AP(dstF_ap.tensor, dstF_ap.offset,
                                          [dstF_ap.ap[0], dstF_ap.ap[1], [0, 128]])
                        iot_ap = iotar_sb[:]
                        iot_bc = bass.AP(iot_ap.tensor, iot_ap.offset,
                                         [iot_ap.ap[0], [0, G], iot_ap.ap[1]])
                        nc.vector.tensor_tensor(out=mask_acc[:, :G, :],
                                                in0=dstF_bc, in1=iot_bc,
                                                op=OP.is_equal)

                        # mask_ne[n, e] = (dst(e) == n), via PE replication
                        mask_ne = ep_.tile([128, GM * 128], bf16, tag="mne")
                        nsl = 0 if "rep" in skip else (Et + 511) // 512
                        if "rep" in skip:
                            nc.vector.memset(mask_ne[:, :Et], 0)
                        for s_ in range(nsl):
                            w = min(512, Et - s_ * 512)
                            psr = ppR.tile([128, 512], fp32, tag="psr")
                            nc.tensor.matmul(
                                out=psr[:, :w], lhsT=ones_sb[:],
                                rhs=dstR_sb[:, s_ * 512:s_ * 512 + w],
                                start=True, stop=True)
                            ic_ap = iotac_sb[:]
                            ic_bc = bass.AP(ic_ap.tensor, ic_ap.offset,
                                            [ic_ap.ap[0], [0, w]])
                            nc.vector.tensor_tensor(
                                out=mask_ne[:, s_ * 512:s_ * 512 + w],
                                in0=psr[:, :w], in1=ic_bc, op=OP.is_equal)

                        # per group: m = xr[dst] + ea@We + xl  (PSUM), then
                        # LeakyRelu on ACT
                        mt = ep_.tile([128, GM, 128], bf16, tag="mt")
                        if "grp" in skip:
                            nc.vector.memset(mt[:, :G, :], 0)
                        else:
                            for g in range(G):
                                psA = ppA.tile([128, 128], fp32, tag="psA")
                                nc.tensor.matmul(
                                    out=psA[:], lhsT=mask_ne[:, g * 128:(g + 1) * 128],
                                    rhs=xr_all[:, t * 256 + d * 128:t * 256 + d * 128 + 128],
                                    start=True, stop=False)
                                nc.tensor.matmul(
                                    out=psA[:], lhsT=eaT_sb[:, g * 128:(g + 1) * 128],
                                    rhs=we_sb[d][:], start=False, stop=False)
                                nc.tensor.matmul(
                                    out=psA[:], lhsT=i128_sb[:], rhs=xlg[:, g, :],
                                    start=False, stop=True)
                                nc.scalar.copy(out=mt[:, g, :], in_=psA[:])
                            # leaky_relu(x) = max(0.2x, x), one DVE op on bf16
                            nc.vector.scalar_tensor_tensor(
                                out=mt[:, :G, :], in0=mt[:, :G, :],
                                scalar=NEG_SLOPE, in1=mt[:, :G, :],
                                op0=OP.mult, op1=OP.max)

                        # score + exp
                        mm = ep_.tile([128, GM, 128], bf16, tag="mm")
                        att_ap = att_sb[d][:]
                        att_bc = bass.AP(att_ap.tensor, att_ap.offset,
                                         [att_ap.ap[0], [0, G], att_ap.ap[1]])
                        nc.vector.tensor_tensor(out=mm[:, :G, :], in0=mt[:, :G, :],
                                                in1=att_bc, op=OP.mult)
                        sc = ep_.tile([128, GM, H], fp32, tag="sc")
                        nc.vector.tensor_reduce(
                            out=sc[:, :G, :],
                            in_=mm[:, :G, :].rearrange("p g (h c) -> p g h c", h=H),
                            axis=AX.X, op=OP.add)
                        av = ep_.tile([128, GM, H], bf16, tag="av")
                        nc.scalar.activation(out=av[:, :G, :], in_=sc[:, :G, :],
                                             func=AF.Exp)

                        # v = [a*xl | a]
                        v = ep_.tile([128, GM, 132], bf16, tag="v")
                        av_ap = av[:, :G, :]
                        av_bc = bass.AP(av_ap.tensor, av_ap.offset,
                                        [av_ap.ap[0], av_ap.ap[1], av_ap.ap[2],
                                         [0, CC]])
                        nc.vector.tensor_tensor(
                            out=v[:, :G, 0:128].rearrange("p g (h c) -> p g h c", h=H),
                            in0=xlg[:, :G, :].rearrange("p g (h c) -> p g h c", h=H),
                            in1=av_bc, op=OP.mult)
                        nc.scalar.copy(out=v[:, :G, 128:132], in_=av[:, :G, :])

                        # segmented accumulation into PSUM
                        nm_sb = fp_.tile([128, 132], fp32, tag=f"nm{d}",
                                         name=f"nm{d}")
                        if "acc" in skip:
                            nc.vector.memset(nm_sb[:], 0)
                        else:
                            psC = ppC.tile([128, 132], fp32, tag=f"acc{d}",
                                           name=f"acc{d}")
                            for g in range(G):
                                nc.tensor.matmul(
                                    out=psC[:], lhsT=mask_acc[:, g, :],
                                    rhs=v[:, g, :], start=(g == 0),
                                    stop=(g == G - 1))
                            nc.scalar.copy(out=nm_sb[:], in_=psC[:])
                        nm[d] = nm_sb

                    # --- combine both directions, write own rows ----------
                    rows = min(128, cfg.NPC - t * 128)
                    acc = fp_.tile([128, HC], fp32, tag="acc")
                    nc.vector.tensor_copy(out=acc[:], in_=bias_sb[:])
                    for d in range(2):
                        num_sb = nm[d][:, 0:128]
                        den_sb = nm[d][:, 128:132]
                        d2 = fp_.tile([128, H], fp32, tag="d2")
                        nc.vector.tensor_scalar(out=d2[:], in0=den_sb,
                                                scalar1=2.0, scalar2=EPS2,
                                                op0=OP.mult, op1=OP.add)
                        rd = fp_.tile([128, H], fp32, tag="rd")
                        nc.vector.reciprocal(out=rd[:], in_=d2[:])
                        rd_ap = rd[:]
                        rd_bc = bass.AP(rd_ap.tensor, rd_ap.offset,
                                        [rd_ap.ap[0], rd_ap.ap[1], [0, CC]])
                        t_d = fp_.tile([128, HC], fp32, tag="t_d")
                        nc.vector.tensor_tensor(
                            out=t_d[:].rearrange("p (h c) -> p h c", h=H),
                            in0=num_sb.rearrange("p (h c) -> p h c", h=H),
                            in1=rd_bc, op=OP.mult)
                        nc.vector.tensor_add(out=acc[:], in0=acc[:], in1=t_d[:])
                        ds = fp_.tile([128, H], fp32, tag="ds")
                        nc.vector.tensor_mul(out=ds[:], in0=den_sb, in1=rd[:])
                        ds_ap = ds[:]
                        ds_bc = bass.AP(ds_ap.tensor, ds_ap.offset,
                                        [ds_ap.ap[0], ds_ap.ap[1], [0, CC]])
                        u_d = fp_.tile([128, HC], fp32, tag="u_d")
                        nc.vector.tensor_tensor(
                            out=u_d[:].rearrange("p (h c) -> p h c", h=H),
                            in0=bl_sb[d][:].rearrange("p (h c) -> p h c", h=H),
                            in1=ds_bc, op=OP.mult)
                        nc.vector.tensor_add(out=acc[:], in0=acc[:], in1=u_d[:])
                    nc.sync.dma_start(out=out_t[t * 128:t * 128 + rows, :],
                                      in_=acc[:rows, :])

    nc.compile()
    return nc


# ---------------------------------------------------------------------------
# entry point
# ---------------------------------------------------------------------------

def kernel(**inputs):
    for p in ("/opt/trn_rl_repo",):
        if p not in sys.path:
            sys.path.insert(0, p)
    from concourse.bass_utils import run_bass_kernel_spmd

    cfg = Cfg()
    shards, cfg = prep_shards(inputs, cfg, NCORES)
    nc = build_program(cfg)
    res = run_bass_kernel_spmd(nc, shards, core_ids=list(range(NCORES)))
    out = np.concatenate([res.results[c]["out"] for c in range(NCORES)], axis=0)
    return out.astype(np.float32)
